# revision 44
# baseline (speedup 1.0000x reference)
"""Trainium2 Bass kernel for nn_Attention_5927054869144.

Channel-attention over [B=8, C=64, H=256, W=256] inputs. Data-parallel over
batch: one batch element per NeuronCore (8 cores), no collectives.

Per-core pipeline (x_b viewed as [64, 65536], spatial blocks of 8192):
  1. f16 input (host-converted), host-restacked slab-major so each load is
     one contiguous [65, 8192] 1MB DMA (double-buffered, alternating the two
     HWDGE queues); ~3.4us of junk matmuls at start latch the PE clock gate
     to 8/8 before the first projection.
  2. qkvT projection with x-chunk stationary on the PE -> q/k/v in
     spatial-partition layout (fp16 operands, fp32 PSUM); PSUM evacuations
     alternate Vector/Scalar (the only PSUM-capable engines).
  3. Per-head-pair dots accumulated in PSUM over all spatial tiles (softmax
     scale folded into Wq/bq host-side); v transposed on the PE into
     dim-partition layout.
  4. Unnormalized softmax (exp with accumulated row sums); 1/rowsum folded
     into per-head copies of Wo^T; per-pair M matrices written into a
     BLOCK-DIAGONAL [128,128] stationary so the final matmul computes both
     heads of a pair in one pass; junk matmuls keep the PE warm through the
     softmax serial chain.
  5. Final output = blockdiag(M)^T @ v_dp in 512-col chunks, evacuated to
     f16 and DMAed as full-128-partition 1MB chunks across 3 DMA paths.
Output returned f16 [128, 32768] per core, unpacked + upcast on host.

NOTE: dma_start_transpose (xbar) for the v-transpose and packing multiple
PSUM accumulation regions into one bank were both tried and caused
nondeterministic corruption on HW; PE-transpose + one-bank-per-region is
the stable configuration.
"""

import os
import sys

import numpy as np

for _p in ("/opt/trn_rl_repo", "/root/.axon_site/_ro/trn_rl_repo"):
    if os.path.isdir(_p) and _p not in sys.path:
        sys.path.insert(0, _p)

from concourse import bacc, mybir, tile  # noqa: E402
from concourse import bass_utils as _bu  # noqa: E402
from concourse.bass_utils import run_bass_kernel_spmd  # noqa: E402

del _bu

F32 = mybir.dt.float32
F16 = mybir.dt.float16

HEADS = 8
C = 64
HW = 65536          # 256*256 spatial positions per batch element
BL = HW // HEADS    # 8192, per-head block length
NSLAB = 8           # input slabs (within-block n ranges)
N_GROUPS = BL // 128  # 64 total tile groups

LAST_RESULTS = None


def _build_kernel(hw=HW):
    bl = hw // HEADS
    n_groups = bl // 128
    chunk_b = min(4096, bl)
    s5n = chunk_b // 512
    c0n = bl // chunk_b

    nc = bacc.Bacc("TRN2", target_bir_lowering=False, debug=False)
    x_d = nc.dram_tensor("x", [65, hw], F16, kind="ExternalInput")
    wqkv_d = nc.dram_tensor("wqkv", [65, 192], F16, kind="ExternalInput")
    wot_d = nc.dram_tensor("wot", [128, 64], F32, kind="ExternalInput")
    ident_d = nc.dram_tensor("ident", [128, 128], F16, kind="ExternalInput")
    out_d = nc.dram_tensor("out", [128, hw // 2], F16, kind="ExternalOutput")

    x_ap = x_d.ap()
    out_ap = out_d.ap()

    with tile.TileContext(nc) as tc:
        with (
            tc.tile_pool(name="consts", bufs=1) as cpool,
            tc.tile_pool(name="pers", bufs=1) as pers,
            tc.tile_pool(name="dotsp", bufs=1, space="PSUM") as dotspool,
        ):
            wqkv_sb = cpool.tile([65, 192], F16)
            wot_sb = cpool.tile([128, 64], F32)
            ident_sb = cpool.tile([128, 128], F16)
            nc.gpsimd.dma_start(out=wqkv_sb[:, :], in_=wqkv_d.ap()[:, :])
            nc.gpsimd.dma_start(out=wot_sb[:, :], in_=wot_d.ap()[:, :])
            nc.gpsimd.dma_start(out=ident_sb[:, :], in_=ident_d.ap()[:, :])

            # v in dim-partition layout: [pair, d(0:64 even head / 64:128 odd), n]
            vdp = pers.tile([128, 4 * bl], F16)
            # block-diagonal per-pair M matrices for the final matmul
            mh_bd = pers.tile([128, 4 * 128], F16)
            nc.vector.memset(mh_bd[:, :], 0.0)
            dots_ps = [
                dotspool.tile([128, 128], F32, name=f"dots{p}") for p in range(4)
            ]

            def evac(i, dst, src):
                # PSUM-sourced copies: only Vector/Scalar can read PSUM
                if i % 2 == 0:
                    nc.vector.tensor_copy(dst, src)
                else:
                    nc.scalar.copy(dst, src)

            # ---------------- Phase A ----------------
            vdp_v = vdp.rearrange("p (r n) -> p r n", r=4)
            ev = 0
            with (
                tc.tile_pool(name="xq", bufs=3) as xpool,
                tc.tile_pool(name="slots", bufs=6) as slotpool,
                tc.tile_pool(name="projp", bufs=3, space="PSUM") as projpool,
                tc.tile_pool(name="vtrp", bufs=1, space="PSUM") as vtrpool,
            ):
                slots = {}

                def consume(g):
                    nonlocal ev
                    # dots + v-transpose for a group whose slot is fully evac'd
                    slot = slots.pop(g)
                    vt = vtrpool.tile([128, 512], F16, name="vt")
                    for pr in range(4):
                        qs = slot[:, 128 * pr: 128 * pr + 128]
                        ks = slot[:, 512 + 128 * pr: 512 + 128 * pr + 128]
                        vs = slot[:, 1024 + 128 * pr: 1024 + 128 * pr + 128]
                        nc.tensor.matmul(
                            dots_ps[pr][:, :],
                            lhsT=qs,
                            rhs=ks,
                            start=(g == 0),
                            stop=(g == n_groups - 1),
                        )
                        nc.tensor.transpose(
                            vt[:, pr * 128:(pr + 1) * 128], vs, ident_sb[:, :]
                        )
                    voff = g * 128
                    evac(ev, vdp_v[:, :, voff:voff + 128], vt[:, :])
                    ev += 1

                # ~3.4us of back-to-back junk matmuls at start: latch the
                # HAM clock gate to 8/8 before the first real projection
                for jt in range(10):
                    jpp = projpool.tile([128, 384], F32, name="pp")
                    for c in range(3):
                        nc.tensor.matmul(
                            jpp[:, c * 128:(c + 1) * 128],
                            lhsT=ident_sb[:, :], rhs=ident_sb[:, :],
                            start=True, stop=True,
                        )

                in_engs = [nc.sync, nc.scalar]
                sl = bl // NSLAB
                tps = sl // 128
                for e in range(NSLAB):
                    xq = xpool.tile([65, 8 * sl], F16, name="xq")
                    # host layout is slab-major: one contiguous 1MB slab
                    in_engs[e % 2].dma_start(
                        out=xq[:, :],
                        in_=x_ap[:, e * 8 * sl:(e + 1) * 8 * sl],
                    )
                    for t0 in range(tps):
                        g = e * tps + t0
                        # slot cols: r*512 + head*64 + i*8 + alpha (alpha contiguous)
                        slot = slotpool.tile([128, 1536], F16, name="slot")
                        slot_sc = slot.rearrange(
                            "p (r h i a) -> p i r h a", r=3, h=8, i=8, a=8
                        )
                        slots[g] = slot
                        for ip in range(4):  # chunk pairs (2*ip, 2*ip+1)
                            pp = projpool.tile([128, 384], F32, name="pp")
                            for c in range(2):
                                i = 2 * ip + c
                                nc.tensor.matmul(
                                    pp[:, c * 192:(c + 1) * 192],
                                    lhsT=xq[:, i * sl + t0 * 128:
                                            i * sl + t0 * 128 + 128],
                                    rhs=wqkv_sb[:, :],
                                    start=True,
                                    stop=True,
                                )
                            evac(ev, slot_sc[:, 2 * ip: 2 * ip + 2, :, :, :],
                                 pp[:, :])
                            ev += 1
                        if g >= 3:
                            consume(g - 3)
                for g in range(n_groups - 3, n_groups):
                    consume(g)

            # ---------------- Softmax + output ----------------
            with (
                tc.tile_pool(name="smx", bufs=1) as smx,
                tc.tile_pool(name="mhp", bufs=1, space="PSUM") as mhpool,
                tc.tile_pool(name="finp", bufs=3, space="PSUM") as finpool,
                tc.tile_pool(name="outs", bufs=3) as outpool,
            ):
                negmax = smx.tile([128, 4], F32)
                rowsum = smx.tile([128, 4], F32)
                recip = smx.tile([128, 4], F32)
                exps = smx.tile([128, 4 * 64], F16)
                wots = smx.tile([128, 4 * 64], F16)
                mh_ps = mhpool.tile([128, 64], F32)

                # keep the PE's HAM activity window busy through the
                # softmax serial chain so the final matmuls run warm
                for _ in range(3):
                    nc.tensor.matmul(
                        mh_ps[:, :], lhsT=ident_sb[:, :],
                        rhs=ident_sb[:, 0:64], start=True, stop=True,
                    )
                for h in range(HEADS):
                    b = (h % 2) * 64
                    pr = h // 2
                    if h % 2 == 0:
                        for _ in range(2):
                            nc.tensor.matmul(
                                mh_ps[:, :], lhsT=ident_sb[:, :],
                                rhs=ident_sb[:, 0:64], start=True, stop=True,
                            )
                    dsl = dots_ps[pr][b:b + 64, b:b + 64]
                    nc.vector.reduce_max(
                        negmax[b:b + 64, pr:pr + 1], dsl,
                        axis=mybir.AxisListType.X, negate=True,
                    )
                    nc.scalar.activation(
                        exps[b:b + 64, pr * 64:(pr + 1) * 64], dsl,
                        mybir.ActivationFunctionType.Exp,
                        bias=negmax[b:b + 64, pr:pr + 1],
                        scale=1.0,
                        accum_out=rowsum[b:b + 64, pr:pr + 1],
                    )
                    nc.vector.reciprocal(
                        recip[b:b + 64, pr:pr + 1], rowsum[b:b + 64, pr:pr + 1]
                    )
                    nc.vector.tensor_scalar_mul(
                        wots[b:b + 64, pr * 64:(pr + 1) * 64],
                        wot_sb[b:b + 64, :],
                        recip[b:b + 64, pr:pr + 1],
                    )
                    nc.tensor.matmul(
                        mh_ps[b:b + 64, :],
                        lhsT=exps[b:b + 64, pr * 64:(pr + 1) * 64],
                        rhs=wots[b:b + 64, pr * 64:(pr + 1) * 64],
                        start=True,
                        stop=True,
                    )
                    # write into the block-diagonal stationary for pair pr
                    nc.vector.tensor_copy(
                        mh_bd[b:b + 64, pr * 128 + b: pr * 128 + b + 64],
                        mh_ps[b:b + 64, :],
                    )

                dma_engs = [nc.sync, nc.scalar, nc.gpsimd]
                ci = 0
                for pr in range(4):
                    for c0 in range(c0n):
                        outsb = outpool.tile([128, chunk_b], F16, name="outsb")
                        for s5 in range(s5n):
                            fp_ = finpool.tile([128, 512], F32, name="fp_")
                            n0 = pr * bl + c0 * chunk_b + s5 * 512
                            nc.tensor.matmul(
                                fp_[:, :],
                                lhsT=mh_bd[:, pr * 128:(pr + 1) * 128],
                                rhs=vdp[:, n0:n0 + 512],
                                start=True,
                                stop=True,
                            )
                            evac(ci + s5, outsb[:, s5 * 512:(s5 + 1) * 512],
                                 fp_[:, :])
                        dma_engs[ci % 3].dma_start(
                            out=out_ap[:, pr * bl + c0 * chunk_b:
                                       pr * bl + (c0 + 1) * chunk_b],
                            in_=outsb[:, :],
                        )
                        ci += 1

    nc.compile()
    return nc


_NC_CACHE = {}


def _get_nc(hw=HW):
    if hw not in _NC_CACHE:
        _NC_CACHE[hw] = _build_kernel(hw)
    return _NC_CACHE[hw]


def _host_inputs(Wq, bq, Wk, bk, Wv, bv, Wo):
    scale = 64 ** -0.5
    wqkv = np.zeros((65, 192), np.float16)
    wqkv[:64, 0:64] = (Wq.T * scale).astype(np.float16)
    wqkv[64, 0:64] = (bq * scale).astype(np.float16)
    wqkv[:64, 64:128] = Wk.T.astype(np.float16)
    wqkv[64, 64:128] = bk.astype(np.float16)
    wqkv[:64, 128:192] = Wv.T.astype(np.float16)
    wqkv[64, 128:192] = bv.astype(np.float16)
    # kernel uses c' = i*8 + alpha ordering; original c = alpha*8 + i
    pi = np.array([(c % 8) * 8 + c // 8 for c in range(64)])
    wotp = Wo.T[pi]
    wot = np.concatenate([wotp, wotp], axis=0).astype(np.float32)
    ident = np.eye(128, dtype=np.float16)
    return wqkv, wot, ident


def kernel(x, Wq, bq, Wk, bk, Wv, bv, Wo):
    global LAST_RESULTS
    B = x.shape[0]
    hw = x.shape[2] * x.shape[3]
    sl = hw // HEADS // NSLAB
    nc = _get_nc(hw)
    wqkv, wot, ident = _host_inputs(Wq, bq, Wk, bk, Wv, bv, Wo)

    in_maps = []
    for bidx in range(B):
        # slab-major layout: col = e*(8*sl) + i*sl + n'
        x65 = np.empty((65, NSLAB, 8, sl), np.float16)
        x65[:64] = x[bidx].reshape(64, 8, NSLAB, sl).transpose(0, 2, 1, 3)
        x65[64] = 1.0
        in_maps.append({
            "x": x65.reshape(65, hw), "wqkv": wqkv, "wot": wot, "ident": ident,
        })

    trace = bool(os.environ.get("KERNEL_TRACE"))
    res = run_bass_kernel_spmd(
        nc, in_maps, core_ids=list(range(B)), trace=trace
    )
    LAST_RESULTS = res
    outs = []
    for bidx in range(B):
        r = res.results[bidx]["out"].reshape(2, 64, 4, hw // HEADS)
        outs.append(
            r.transpose(1, 2, 0, 3).reshape(64, HEADS, hw // HEADS)
            .astype(np.float32)
        )
    return np.stack(outs)


# revision 46
# speedup vs baseline: 1.1819x; 1.1819x over previous
"""Trainium2 Bass kernel for nn_Attention_5927054869144.

Channel-attention over [B=8, C=64, H=256, W=256] inputs. Data-parallel over
batch: one batch element per NeuronCore (8 cores), no collectives.

Per-core pipeline (x_b viewed as [64, 65536], spatial blocks of 8192):
  1. f16 input (host-converted), host-restacked slab-major so each load is
     one contiguous [65, 8192] 1MB DMA (double-buffered, alternating the two
     HWDGE queues); ~3.4us of junk matmuls at start latch the PE clock gate
     to 8/8 before the first projection.
  2. qkvT projection with x-chunk stationary on the PE -> q/k/v in
     spatial-partition layout (fp16 operands, fp32 PSUM); PSUM evacuations
     alternate Vector/Scalar (the only PSUM-capable engines).
  3. Per-head-pair dots accumulated in PSUM over all spatial tiles (softmax
     scale folded into Wq/bq host-side); v transposed on the PE into
     dim-partition layout.
  4. Unnormalized softmax (exp with accumulated row sums); 1/rowsum folded
     into per-head copies of Wo^T; per-pair M matrices written into a
     BLOCK-DIAGONAL [128,128] stationary so the final matmul computes both
     heads of a pair in one pass; junk matmuls keep the PE warm through the
     softmax serial chain.
  5. Final output = blockdiag(M)^T @ v_dp in 512-col chunks, evacuated to
     f16 and DMAed as full-128-partition 1MB chunks across 3 DMA paths.
Output returned f16 [128, 32768] per core, unpacked + upcast on host.

NOTE: dma_start_transpose (xbar) for the v-transpose and packing multiple
PSUM accumulation regions into one bank were both tried and caused
nondeterministic corruption on HW; PE-transpose + one-bank-per-region is
the stable configuration.
"""

import os
import sys

import numpy as np

for _p in ("/opt/trn_rl_repo", "/root/.axon_site/_ro/trn_rl_repo"):
    if os.path.isdir(_p) and _p not in sys.path:
        sys.path.insert(0, _p)

from concourse import bacc, mybir, tile  # noqa: E402
from concourse import bass_utils as _bu  # noqa: E402
from concourse.bass_utils import run_bass_kernel_spmd  # noqa: E402

del _bu

F32 = mybir.dt.float32
F16 = mybir.dt.float16

HEADS = 8
C = 64
HW = 65536          # 256*256 spatial positions per batch element
BL = HW // HEADS    # 8192, per-head block length
NSLAB = 8           # input slabs (within-block n ranges)
N_GROUPS = BL // 128  # 64 total tile groups

LAST_RESULTS = None


def _build_kernel(hw=HW):
    bl = hw // HEADS
    n_groups = bl // 128
    chunk_b = min(4096, bl)
    s5n = chunk_b // 512
    c0n = bl // chunk_b

    nc = bacc.Bacc("TRN2", target_bir_lowering=False, debug=False)
    x_d = nc.dram_tensor("x", [65, hw], F16, kind="ExternalInput")
    wqkv_d = nc.dram_tensor("wqkv", [65, 192], F16, kind="ExternalInput")
    wot_d = nc.dram_tensor("wot", [128, 64], F32, kind="ExternalInput")
    ident_d = nc.dram_tensor("ident", [128, 128], F16, kind="ExternalInput")
    out_d = nc.dram_tensor("out", [128, hw // 2], F16, kind="ExternalOutput")

    x_ap = x_d.ap()
    out_ap = out_d.ap()

    with tile.TileContext(nc) as tc:
        with (
            tc.tile_pool(name="consts", bufs=1) as cpool,
            tc.tile_pool(name="pers", bufs=1) as pers,
            tc.tile_pool(name="dotsp", bufs=1, space="PSUM") as dotspool,
        ):
            wqkv_sb = cpool.tile([65, 192], F16)
            wot_sb = cpool.tile([128, 64], F32)
            ident_sb = cpool.tile([128, 128], F16)
            nc.gpsimd.dma_start(out=wqkv_sb[:, :], in_=wqkv_d.ap()[:, :])
            nc.gpsimd.dma_start(out=wot_sb[:, :], in_=wot_d.ap()[:, :])
            nc.gpsimd.dma_start(out=ident_sb[:, :], in_=ident_d.ap()[:, :])

            # v in dim-partition layout: [pair, d(0:64 even head / 64:128 odd), n]
            vdp = pers.tile([128, 4 * bl], F16)
            # block-diagonal per-pair M matrices for the final matmul
            mh_bd = pers.tile([128, 4 * 128], F16)
            nc.vector.memset(mh_bd[:, :], 0.0)
            dots_ps = [
                dotspool.tile([128, 128], F32, name=f"dots{p}") for p in range(4)
            ]

            def evac(i, dst, src):
                # PSUM-sourced copies: only Vector/Scalar can read PSUM
                if i % 2 == 0:
                    nc.vector.tensor_copy(dst, src)
                else:
                    nc.scalar.copy(dst, src)

            # ---------------- Phase A ----------------
            vdp_v = vdp.rearrange("p (r n) -> p r n", r=4)
            ev = 0
            with (
                tc.tile_pool(name="xq", bufs=3) as xpool,
                tc.tile_pool(name="slots", bufs=6) as slotpool,
                tc.tile_pool(name="projp", bufs=3, space="PSUM") as projpool,
                tc.tile_pool(name="vtrp", bufs=1, space="PSUM") as vtrpool,
            ):
                slots = {}

                def consume(g):
                    nonlocal ev
                    # dots + v-transpose for a group whose slot is fully evac'd
                    slot = slots.pop(g)
                    vt = vtrpool.tile([128, 512], F16, name="vt")
                    for pr in range(4):
                        qs = slot[:, 128 * pr: 128 * pr + 128]
                        ks = slot[:, 512 + 128 * pr: 512 + 128 * pr + 128]
                        vs = slot[:, 1024 + 128 * pr: 1024 + 128 * pr + 128]
                        nc.tensor.matmul(
                            dots_ps[pr][:, :],
                            lhsT=qs,
                            rhs=ks,
                            start=(g == 0),
                            stop=(g == n_groups - 1),
                        )
                        nc.tensor.transpose(
                            vt[:, pr * 128:(pr + 1) * 128], vs, ident_sb[:, :]
                        )
                    voff = g * 128
                    evac(ev, vdp_v[:, :, voff:voff + 128], vt[:, :])
                    ev += 1

                # ~3.4us of back-to-back junk matmuls at start: latch the
                # HAM clock gate to 8/8 before the first real projection
                for jt in range(10):
                    jpp = projpool.tile([128, 384], F32, name="pp")
                    for c in range(3):
                        nc.tensor.matmul(
                            jpp[:, c * 128:(c + 1) * 128],
                            lhsT=ident_sb[:, :], rhs=ident_sb[:, :],
                            start=True, stop=True,
                        )

                in_engs = [nc.sync, nc.scalar]
                sl = bl // NSLAB
                tps = sl // 128
                for e in range(NSLAB):
                    xq = xpool.tile([65, 8 * sl], F16, name="xq")
                    # host layout is slab-major: one contiguous 1MB slab
                    in_engs[e % 2].dma_start(
                        out=xq[:, :],
                        in_=x_ap[:, e * 8 * sl:(e + 1) * 8 * sl],
                    )
                    for t0 in range(tps):
                        g = e * tps + t0
                        # slot cols: r*512 + head*64 + i*8 + alpha (alpha contiguous)
                        slot = slotpool.tile([128, 1536], F16, name="slot")
                        slot_sc = slot.rearrange(
                            "p (r h i a) -> p i r h a", r=3, h=8, i=8, a=8
                        )
                        slots[g] = slot
                        for ip in range(4):  # chunk pairs (2*ip, 2*ip+1)
                            pp = projpool.tile([128, 384], F32, name="pp")
                            for c in range(2):
                                i = 2 * ip + c
                                nc.tensor.matmul(
                                    pp[:, c * 192:(c + 1) * 192],
                                    lhsT=xq[:, i * sl + t0 * 128:
                                            i * sl + t0 * 128 + 128],
                                    rhs=wqkv_sb[:, :],
                                    start=True,
                                    stop=True,
                                )
                            evac(ev, slot_sc[:, 2 * ip: 2 * ip + 2, :, :, :],
                                 pp[:, :])
                            ev += 1
                        if g >= 3:
                            consume(g - 3)
                for g in range(n_groups - 3, n_groups):
                    consume(g)

            # ---------------- Softmax + output ----------------
            with (
                tc.tile_pool(name="smx", bufs=1) as smx,
                tc.tile_pool(name="mhp", bufs=1, space="PSUM") as mhpool,
                tc.tile_pool(name="finp", bufs=3, space="PSUM") as finpool,
                tc.tile_pool(name="outs", bufs=3) as outpool,
            ):
                negmax = smx.tile([128, 4], F32)
                rowsum = smx.tile([128, 4], F32)
                recip = smx.tile([128, 4], F32)
                exps = smx.tile([128, 4 * 64], F16)
                wots = smx.tile([128, 4 * 64], F16)
                mh_ps = mhpool.tile([128, 64], F32)

                # keep the PE's HAM activity window busy through the
                # softmax serial chain so the final matmuls run warm
                for _ in range(3):
                    nc.tensor.matmul(
                        mh_ps[:, :], lhsT=ident_sb[:, :],
                        rhs=ident_sb[:, 0:64], start=True, stop=True,
                    )
                for h in range(HEADS):
                    b = (h % 2) * 64
                    pr = h // 2
                    if h % 2 == 0:
                        for _ in range(2):
                            nc.tensor.matmul(
                                mh_ps[:, :], lhsT=ident_sb[:, :],
                                rhs=ident_sb[:, 0:64], start=True, stop=True,
                            )
                    dsl = dots_ps[pr][b:b + 64, b:b + 64]
                    nc.vector.reduce_max(
                        negmax[b:b + 64, pr:pr + 1], dsl,
                        axis=mybir.AxisListType.X, negate=True,
                    )
                    nc.scalar.activation(
                        exps[b:b + 64, pr * 64:(pr + 1) * 64], dsl,
                        mybir.ActivationFunctionType.Exp,
                        bias=negmax[b:b + 64, pr:pr + 1],
                        scale=1.0,
                        accum_out=rowsum[b:b + 64, pr:pr + 1],
                    )
                    nc.vector.reciprocal(
                        recip[b:b + 64, pr:pr + 1], rowsum[b:b + 64, pr:pr + 1]
                    )
                    nc.vector.tensor_scalar_mul(
                        wots[b:b + 64, pr * 64:(pr + 1) * 64],
                        wot_sb[b:b + 64, :],
                        recip[b:b + 64, pr:pr + 1],
                    )
                    nc.tensor.matmul(
                        mh_ps[b:b + 64, :],
                        lhsT=exps[b:b + 64, pr * 64:(pr + 1) * 64],
                        rhs=wots[b:b + 64, pr * 64:(pr + 1) * 64],
                        start=True,
                        stop=True,
                    )
                    # write into the block-diagonal stationary for pair pr
                    nc.vector.tensor_copy(
                        mh_bd[b:b + 64, pr * 128 + b: pr * 128 + b + 64],
                        mh_ps[b:b + 64, :],
                    )

                dma_engs = [nc.sync, nc.scalar, nc.gpsimd]
                ci = 0
                for pr in range(4):
                    for c0 in range(c0n):
                        outsb = outpool.tile([128, chunk_b], F16, name="outsb")
                        for s5 in range(s5n):
                            fp_ = finpool.tile([128, 512], F32, name="fp_")
                            n0 = pr * bl + c0 * chunk_b + s5 * 512
                            nc.tensor.matmul(
                                fp_[:, :],
                                lhsT=mh_bd[:, pr * 128:(pr + 1) * 128],
                                rhs=vdp[:, n0:n0 + 512],
                                start=True,
                                stop=True,
                            )
                            evac(ci + s5, outsb[:, s5 * 512:(s5 + 1) * 512],
                                 fp_[:, :])
                        dma_engs[ci % 3].dma_start(
                            out=out_ap[:, pr * bl + c0 * chunk_b:
                                       pr * bl + (c0 + 1) * chunk_b],
                            in_=outsb[:, :],
                        )
                        ci += 1

    nc.compile()
    return nc


_NC_CACHE = {}


def _get_nc(hw=HW):
    if hw not in _NC_CACHE:
        _NC_CACHE[hw] = _build_kernel(hw)
    return _NC_CACHE[hw]


def _host_inputs(Wq, bq, Wk, bk, Wv, bv, Wo):
    scale = 64 ** -0.5
    wqkv = np.zeros((65, 192), np.float16)
    wqkv[:64, 0:64] = (Wq.T * scale).astype(np.float16)
    wqkv[64, 0:64] = (bq * scale).astype(np.float16)
    wqkv[:64, 64:128] = Wk.T.astype(np.float16)
    wqkv[64, 64:128] = bk.astype(np.float16)
    wqkv[:64, 128:192] = Wv.T.astype(np.float16)
    wqkv[64, 128:192] = bv.astype(np.float16)
    # kernel uses c' = i*8 + alpha ordering; original c = alpha*8 + i
    pi = np.array([(c % 8) * 8 + c // 8 for c in range(64)])
    wotp = Wo.T[pi]
    wot = np.concatenate([wotp, wotp], axis=0).astype(np.float32)
    ident = np.eye(128, dtype=np.float16)
    return wqkv, wot, ident


def kernel(x, Wq, bq, Wk, bk, Wv, bv, Wo):
    global LAST_RESULTS
    B = x.shape[0]
    hw = x.shape[2] * x.shape[3]
    sl = hw // HEADS // NSLAB
    nc = _get_nc(hw)
    wqkv, wot, ident = _host_inputs(Wq, bq, Wk, bk, Wv, bv, Wo)

    in_maps = []
    for bidx in range(B):
        # slab-major layout: col = e*(8*sl) + i*sl + n'
        x65 = np.empty((65, NSLAB, 8, sl), np.float16)
        x65[:64] = x[bidx].reshape(64, 8, NSLAB, sl).transpose(0, 2, 1, 3)
        x65[64] = 1.0
        in_maps.append({
            "x": x65.reshape(65, hw), "wqkv": wqkv, "wot": wot, "ident": ident,
        })

    trace = bool(os.environ.get("KERNEL_TRACE"))
    res = run_bass_kernel_spmd(
        nc, in_maps, core_ids=list(range(B)), trace=trace
    )
    LAST_RESULTS = res
    outs = []
    for bidx in range(B):
        r = res.results[bidx]["out"].reshape(2, 64, 4, hw // HEADS)
        outs.append(
            r.transpose(1, 2, 0, 3).reshape(64, HEADS, hw // HEADS)
            .astype(np.float32)
        )
    return np.stack(outs)
